# revision 13
# baseline (speedup 1.0000x reference)
"""Compositional attention Trainium2 Bass kernel (V3: HAM-warm schedule).

Sharding: 8 cores = 2 batches x 4 search-pairs.  Core c handles batch
b=c//4 and searches (2*(c%4), 2*(c%4)+1); each core produces a partial
output for its 128 rows of the S*D=512 concat dim (host sums 4 partials
per batch).

V3 notes (vs V2, 363us):
  - Wrk is folded into Wrq on the HOST (r_sim = sum_d (rq@WrkT)*retrieved),
    removing every fp32 matmul (4 cyc/row) from the device kernel.
  - x is DMA'd in 8 k-chunks and projections are ordered k,q,v,rq so the
    PE stream starts at ~3us and never gaps >3.4us: the HAM clock gate
    (K=4/8 = 1.2GHz default) un-throttles to 2.4GHz and should stay there.
  - Attention runs per-search with a software-pipelined score->exp->
    retrieve schedule (retrieval of key-pair j issues under scores of
    j+1) across an interleaved pair of query blocks.
  - Softmax denominators: flat-2D bf16 add tree (targets DVE 2x mode)
    + one ones-matmul per (si, ib); the r-softmax diff uses the folded
    rqw via one elementwise mult + a signed-ones matmul.  Both stream
    to DRAM per-ib during attention so the epilogue is short.
  - Combine uses a stacked-identity matmul whose output lands directly
    on the search's partition half (tile_position inferred), no
    partition-shift DMAs.
"""

import sys

for _p in ("/opt/trn_rl_repo",):
    if _p not in sys.path:
        sys.path.insert(0, _p)

from contextlib import ExitStack

import ml_dtypes
import numpy as np

import concourse.bass as bass
import concourse.tile as tile
from concourse import bacc
from concourse import mybir
from concourse.bass import ts
from concourse.bass_utils import run_bass_kernel_spmd
from concourse.masks import make_identity

B, N, DIM, S, R, D = 2, 2048, 1024, 8, 2, 64
NCORES = 8
SPC = 2          # searches per core
SD = SPC * D     # 128 (per-core slice of S*D)
RD = R * D       # 128
P = 128
IBL = 512        # i-block (query block)
NIB = N // IBL   # 4
KC = DIM // P    # 8
NJT = N // P     # 16 key tiles
F32 = mybir.dt.float32
BF16 = mybir.dt.bfloat16
SCALE = float(D) ** -0.5
AF = mybir.ActivationFunctionType
ALU = mybir.AluOpType


def _emit(ctx: ExitStack, tc: tile.TileContext, io):
    nc = tc.nc
    xT, wq, wk, wr, wv, wout, outp = io

    singles = ctx.enter_context(tc.tile_pool(name="singles", bufs=1))
    ident = singles.tile([P, P], BF16)
    make_identity(nc, ident)
    ones_b = singles.tile([P, 1], BF16)
    nc.vector.memset(ones_b, 1.0)
    sones = singles.tile([P, 1], BF16)
    nc.vector.memset(sones[0:64, :], 1.0)
    nc.vector.memset(sones[64:128, :], -1.0)
    # stacked identity [I64; I64] for the r-combine partition fold
    stackI = singles.tile([P, 64], BF16)
    nc.vector.tensor_tensor(stackI, ident[:, 0:64], ident[:, 64:128], ALU.add)

    wk_sb = singles.tile([P, KC, SD], BF16)
    wq_sb = singles.tile([P, KC, SD], BF16)
    wv_sb = singles.tile([P, KC, RD], BF16)
    wr_sb = singles.tile([P, KC, SD], BF16)
    # contiguous per-k-chunk weight loads, spread over issue queues so the
    # first projection matmul's deps land ASAP
    wqueues = (nc.gpsimd, nc.scalar, nc.sync)
    for wi, (dst, src) in enumerate(
            ((wk_sb, wk), (wq_sb, wq), (wv_sb, wv), (wr_sb, wr))):
        for k in range(KC):
            wqueues[wi % 3].dma_start(out=dst[:, k, :], in_=src[ts(k, P), :])
    wout_sb = singles.tile([P, DIM], BF16)
    nc.scalar.dma_start(out=wout_sb, in_=wout)

    acts = ctx.enter_context(tc.tile_pool(name="acts", bufs=1))
    qT = acts.tile([P, N], BF16)
    kT = acts.tile([P, N], BF16)
    vT = acts.tile([P, N], BF16)
    rqT = acts.tile([P, N], BF16)
    vnat = acts.tile([P, NJT, RD], BF16)   # [key-part, key-tile, r*d]
    ret0 = acts.tile([P, N], BF16)         # search0 retrievedT (unnormalized)
    ret1 = acts.tile([P, N], BF16)
    rqf0 = acts.tile([P, N], BF16)         # rqw of search0 dup'd on both halves
    rqf1 = acts.tile([P, N], BF16)
    red0 = acts.tile([P, N], BF16)         # per-key-tile exp partial sums
    red1 = acts.tile([P, N], BF16)
    comp = acts.tile([P, N], BF16)         # composed output, stacked searches

    rets = (ret0, ret1)
    reds = (red0, red1)
    rqfs = (rqf0, rqf1)

    # ---------------- projections (k, q, v, rq) ----------------
    with tc.tile_pool(name="xpool", bufs=1) as xpool, \
         tc.tile_pool(name="ppsum", bufs=1, space="PSUM") as ppsum:
        xs = xpool.tile([P, KC, N], BF16)
        for k in range(KC):
            nc.sync.dma_start(out=xs[:, k, :], in_=xT[ts(k, P), :])
        # 8 psum names = 2 generations of 4 chains: dest d+1's chains never
        # wait on dest d's psum->sbuf copies
        pss8 = [ppsum.tile([P, IBL], F32, tag="pj", name=f"pj{j}")
                for j in range(8)]
        for di, (wsb, dest) in enumerate(
                ((wk_sb, kT), (wq_sb, qT), (wv_sb, vT), (wr_sb, rqT))):
            pss = pss8[4 * (di % 2):4 * (di % 2) + 4]
            for k in range(KC):
                for ib in range(NIB):
                    nc.tensor.matmul(
                        pss[ib],
                        lhsT=wsb[:, k, :],
                        rhs=xs[:, k, ts(ib, IBL)],
                        start=(k == 0),
                        stop=(k == KC - 1),
                    )
            for ib in range(NIB):
                nc.vector.tensor_copy(out=dest[:, ts(ib, IBL)], in_=pss[ib])
                if dest is vT:
                    # v to natural [keys, r*d] layout via DMA transposes
                    # (keeps the PE stream matmul-only)
                    for h in range(4):
                        jt = 4 * ib + h
                        tq = nc.scalar if jt % 2 == 0 else nc.sync
                        tq.dma_start_transpose(
                            out=vnat[:, jt, :], in_=vT[:, ts(jt, P)])
        # duplicate each search's rqw onto both partition halves
        for si in range(SPC):
            lo = 64 * si
            nc.gpsimd.dma_start(out=rqfs[si][0:64, :],
                                in_=rqT[lo:lo + 64, :])
            nc.gpsimd.dma_start(out=rqfs[si][64:128, :],
                                in_=rqT[lo:lo + 64, :])

    # DRAM bounce buffers for per-query scalars ([1,N] <-> [128,N/128])
    dramp = ctx.enter_context(tc.tile_pool(name="dramp", bufs=1, space="DRAM"))
    diff_dr = [dramp.tile([N], F32, tag=f"diff{si}", name=f"diff{si}")
               for si in range(SPC)]
    sums_dr = [dramp.tile([N], F32, tag=f"sums{si}", name=f"sums{si}")
               for si in range(SPC)]
    ra0_dr = [dramp.tile([N], BF16, tag=f"ra0{si}", name=f"ra0d{si}")
              for si in range(SPC)]
    ra1_dr = [dramp.tile([N], BF16, tag=f"ra1{si}", name=f"ra1d{si}")
              for si in range(SPC)]

    # pools that live from attention to the end
    epsum = ctx.enter_context(tc.tile_pool(name="epsum", bufs=2, space="PSUM"))
    etmp = ctx.enter_context(tc.tile_pool(name="etmp", bufs=2))
    btmp = ctx.enter_context(tc.tile_pool(name="btmp", bufs=1))

    dmults = {}

    def sums_diff_emit(si, ib):
        """Partition-reduce the exp sums and the rqw*ret diff for one
        (search, query-block); stream both to DRAM.  The elementwise
        dmult was computed at pair-end so the matmuls here have
        long-ready deps and never stall the in-order PE stream."""
        psm = epsum.tile([P, IBL], F32, tag="ep", name="ep")
        nc.tensor.matmul(psm[:1, :], lhsT=ones_b,
                         rhs=reds[si][:, ts(ib, IBL)], start=True, stop=True)
        smst = etmp.tile([1, IBL], F32, tag="smst")
        nc.scalar.copy(out=smst, in_=psm[:1, :])
        nc.sync.dma_start(out=sums_dr[si][None, ts(ib, IBL)], in_=smst[0:1, :])

        pdm = epsum.tile([P, IBL], F32, tag="ep", name="ep")
        nc.tensor.matmul(pdm[:1, :], lhsT=sones, rhs=dmults[(si, ib)],
                         start=True, stop=True)
        pdst = etmp.tile([1, IBL], F32, tag="pdst")
        nc.scalar.copy(out=pdst, in_=pdm[:1, :])
        nc.sync.dma_start(out=diff_dr[si][None, ts(ib, IBL)], in_=pdst[0:1, :])

    def epilogue_emit(si):
        """r-softmax weights for one search -> broadcast -> combined
        output rows land on comp[64*si : 64*si+64]."""
        d128 = etmp.tile([P, N // P], F32, tag="d128")
        s128 = etmp.tile([P, N // P], F32, tag="s128")
        nc.gpsimd.dma_start(out=d128,
                            in_=diff_dr[si].rearrange("(p f) -> p f", p=P))
        nc.gpsimd.dma_start(out=s128,
                            in_=sums_dr[si].rearrange("(p f) -> p f", p=P))
        inv = etmp.tile([P, N // P], F32, tag="inv")
        nc.vector.reciprocal(inv, s128)
        targ = etmp.tile([P, N // P], F32, tag="targ")
        nc.vector.tensor_tensor(targ, d128, inv, ALU.mult)
        sig = etmp.tile([P, N // P], F32, tag="sig")
        nc.scalar.activation(out=sig, in_=targ, func=AF.Sigmoid, scale=SCALE)
        isig = etmp.tile([P, N // P], F32, tag="isig")
        nc.scalar.activation(out=isig, in_=targ, func=AF.Sigmoid, scale=-SCALE)
        ra0 = etmp.tile([P, N // P], BF16, tag="ra0")
        nc.vector.tensor_tensor(ra0, sig, inv, ALU.mult)
        ra1 = etmp.tile([P, N // P], BF16, tag="ra1")
        nc.vector.tensor_tensor(ra1, isig, inv, ALU.mult)
        nc.gpsimd.dma_start(out=ra0_dr[si].rearrange("(p f) -> p f", p=P),
                            in_=ra0)
        nc.gpsimd.dma_start(out=ra1_dr[si].rearrange("(p f) -> p f", p=P),
                            in_=ra1)
        rab = btmp.tile([P, N], BF16, tag=f"rab{si}", name=f"rab{si}")
        nc.gpsimd.dma_start(out=rab[0:64, :],
                            in_=ra0_dr[si][None, :].to_broadcast([64, N]))
        nc.gpsimd.dma_start(out=rab[64:128, :],
                            in_=ra1_dr[si][None, :].to_broadcast([64, N]))
        tmp = btmp.tile([P, N], BF16, tag=f"tmp{si}", name=f"tmp{si}")
        nc.vector.tensor_tensor(tmp, rab, rets[si], ALU.mult)
        lo = 64 * si
        for h in range(NIB):
            cp = epsum.tile([P, IBL], F32, tag="ep", name="ep")
            nc.tensor.matmul(cp[lo:lo + 64, :], lhsT=stackI,
                             rhs=tmp[:, ts(h, IBL)], start=True, stop=True)
            nc.scalar.copy(out=comp[lo:lo + 64, ts(h, IBL)],
                           in_=cp[lo:lo + 64, :])

    # ---------------- attention (per search, ib-pairs) ----------------
    with tc.tile_pool(name="expp", bufs=2) as expp, \
         tc.tile_pool(name="trp1", bufs=2) as trp1, \
         tc.tile_pool(name="trp2", bufs=2) as trp2, \
         tc.tile_pool(name="trp3", bufs=2) as trp3, \
         tc.tile_pool(name="scp", bufs=1, space="PSUM") as scp, \
         tc.tile_pool(name="mps", bufs=1, space="PSUM") as mps:
        pending = []       # deferred sums/diff emits to slot into PE stream
        for si in range(SPC):
            lo = 64 * si
            for ibp in range(NIB // 2):
                ibs = (2 * ibp, 2 * ibp + 1)
                ets = {ib: expp.tile([P, NJT, IBL], BF16, tag="exp",
                                     name=f"exp{ib % 2}")
                       for ib in ibs}
                rt = {ib: mps.tile([P, IBL], F32, tag="mm",
                                   name=f"rt{ib % 2}")
                      for ib in ibs}
                for jg in range(NJT // 2 + 1):
                    # mid-pair: emit deferred work so its matmuls slot
                    # into a busy PE stream (deps are long since ready)
                    if jg in (3, 7) and pending:
                        fn, args = pending.pop(0)
                        fn(*args)
                    if si == 1 and ibp == 1 and jg == 3:
                        epilogue_emit(0)
                    for ib in ibs:
                        if jg < NJT // 2:
                            sp = scp.tile([P, 2, IBL], F32, tag="sc",
                                          name=f"sc{ib % 2}")
                            for h in range(2):
                                jt = 2 * jg + h
                                nc.tensor.matmul(
                                    sp[:, h, :],
                                    lhsT=kT[lo:lo + 64, ts(jt, P)],
                                    rhs=qT[lo:lo + 64, ts(ib, IBL)],
                                    start=True, stop=True,
                                )
                            nc.scalar.activation(
                                out=ets[ib][:, ts(jg, 2), :], in_=sp,
                                func=AF.Exp, scale=SCALE,
                            )
                        if jg > 0:
                            for h in range(2):
                                jt = 2 * (jg - 1) + h
                                nc.tensor.matmul(
                                    rt[ib], lhsT=vnat[:, jt, :],
                                    rhs=ets[ib][:, jt, :],
                                    start=(jt == 0), stop=(jt == NJT - 1),
                                    skip_group_check=True,
                                )
                for ib in ibs:
                    nc.vector.tensor_copy(out=rets[si][:, ts(ib, IBL)],
                                          in_=rt[ib])
                    # flat-2D bf16 add tree over the 16 key tiles
                    etf = ets[ib].rearrange("p a b -> p (a b)")
                    g1 = trp1.tile([P, NJT // 2 * IBL], BF16, tag="g1")
                    nc.vector.tensor_tensor(g1, etf[:, 0:4096],
                                            etf[:, 4096:8192], ALU.add)
                    g2 = trp2.tile([P, NJT // 4 * IBL], BF16, tag="g2")
                    nc.vector.tensor_tensor(g2, g1[:, 0:2048],
                                            g1[:, 2048:4096], ALU.add)
                    g3 = trp3.tile([P, NJT // 8 * IBL], BF16, tag="g3")
                    nc.vector.tensor_tensor(g3, g2[:, 0:1024],
                                            g2[:, 1024:2048], ALU.add)
                    nc.gpsimd.tensor_tensor(reds[si][:, ts(ib, IBL)],
                                            g3[:, 0:512], g3[:, 512:1024],
                                            ALU.add)
                    dm = etmp.tile([P, IBL], BF16, tag="dmult",
                                   name=f"dmult{ib % 2}")
                    nc.vector.tensor_tensor(dm, rqfs[si][:, ts(ib, IBL)],
                                            rets[si][:, ts(ib, IBL)],
                                            ALU.mult)
                    dmults[(si, ib)] = dm
                    pending.append((sums_diff_emit, (si, ib)))

    # ---------------- tail: drain + epilogue(1) + output projection ----
    with tc.tile_pool(name="warmp", bufs=1, space="PSUM") as warmp, \
         tc.tile_pool(name="opo", bufs=2, space="PSUM") as opo:
        scratch = warmp.tile([P, IBL], F32, tag="warm")

        def filler(n):
            # redundant matmuls into a never-read scratch bank: keep the
            # HAM activity window busy so the PE doesn't re-throttle
            # while the epilogue's scalar dance resolves
            for f in range(n):
                nc.tensor.matmul(scratch, lhsT=wout_sb[:, 0:P],
                                 rhs=qT[:, ts(f % 4, IBL)],
                                 start=True, stop=True)

        for fn, args in pending:
            fn(*args)
            filler(2)
        filler(8)
        epilogue_emit(1)

        for nch in range(N // P):
            for h in range(DIM // IBL):
                j = 2 * nch + h
                pw = opo.tile([P, IBL], F32, tag="pw", name=f"pw{j % 2}")
                nc.tensor.matmul(pw, lhsT=comp[:, ts(nch, P)],
                                 rhs=wout_sb[:, ts(h, IBL)],
                                 start=True, stop=True)
                owst = etmp.tile([P, IBL], F32, tag="owst",
                                 name=f"owst{j % 2}")
                if j % 2 == 0:
                    nc.scalar.copy(out=owst, in_=pw)
                else:
                    nc.vector.tensor_copy(out=owst, in_=pw)
                wqueues[j % 3].dma_start(out=outp[ts(nch, P), ts(h, IBL)],
                                         in_=owst)


def build_nc():
    nc = bacc.Bacc()
    xT = nc.declare_dram_parameter("xT", [DIM, N], BF16, isOutput=False)
    wq = nc.declare_dram_parameter("wq", [DIM, SD], BF16, isOutput=False)
    wk = nc.declare_dram_parameter("wk", [DIM, SD], BF16, isOutput=False)
    wr = nc.declare_dram_parameter("wr", [DIM, SD], BF16, isOutput=False)
    wv = nc.declare_dram_parameter("wv", [DIM, RD], BF16, isOutput=False)
    wout = nc.declare_dram_parameter("wout", [SD, DIM], BF16, isOutput=False)
    outp = nc.declare_dram_parameter("outp", [N, DIM], F32, isOutput=True)
    io = (xT[:], wq[:], wk[:], wr[:], wv[:], wout[:], outp[:])
    with tile.TileContext(nc) as tc:
        with ExitStack() as ctx:
            _emit(ctx, tc, io)
    nc.compile()
    return nc


_CACHE = {}


def _get_nc():
    if "nc" not in _CACHE:
        _CACHE["nc"] = build_nc()
    return _CACHE["nc"]


def make_in_maps(x, Wsq, Wsk, Wrv, Wrq, Wrk, Wout):
    x = np.asarray(x, np.float32)
    bf = ml_dtypes.bfloat16
    # fold Wrk into Wrq:  r_sim = sum_d (rq @ Wrk^T) * retrieved
    wr_folded = (
        np.asarray(Wrq, np.float32).reshape(DIM, S, D)
        @ np.asarray(Wrk, np.float32).T
    ).reshape(DIM, S * D)
    in_maps = []
    for c in range(NCORES):
        b = c // 4
        s0 = 2 * (c % 4)
        sl = slice(s0 * D, (s0 + 2) * D)
        in_maps.append({
            "xT": np.ascontiguousarray(x[b].T).astype(bf),
            "wq": np.ascontiguousarray(np.asarray(Wsq, np.float32)[:, sl]).astype(bf),
            "wk": np.ascontiguousarray(np.asarray(Wsk, np.float32)[:, sl]).astype(bf),
            "wr": np.ascontiguousarray(wr_folded[:, sl]).astype(bf),
            "wv": np.ascontiguousarray(np.asarray(Wrv, np.float32)).astype(bf),
            "wout": np.ascontiguousarray(np.asarray(Wout, np.float32)[sl, :]).astype(bf),
        })
    return in_maps


def combine(results):
    out = np.zeros((B, N, DIM), np.float32)
    for c in range(NCORES):
        out[c // 4] += np.asarray(results[c]["outp"], np.float32)
    return out


def kernel(x, Wsq, Wsk, Wrv, Wrq, Wrk, Wout):
    nc = _get_nc()
    in_maps = make_in_maps(x, Wsq, Wsk, Wrv, Wrq, Wrk, Wout)
    res = run_bass_kernel_spmd(nc, in_maps, list(range(NCORES))).results
    return combine(res)


def _install_ntff_shim():
    """Provide antenv.axon_hooks in images that lack it, driving NTFF
    profiling via ctypes into the injected libaxon_pjrt.so."""
    import types
    import ctypes
    import contextlib

    try:
        from antenv.axon_hooks import get_axon_ntff_profile_hook  # noqa
        return
    except ImportError:
        pass
    so_path = "/opt/axon/libaxon_pjrt.so"
    lib = ctypes.CDLL(so_path)
    if not hasattr(lib, "axon_start_nrt_profile"):
        return
    lib.axon_start_nrt_profile.argtypes = [
        ctypes.POINTER(ctypes.c_int64), ctypes.c_size_t]
    lib.axon_start_nrt_profile.restype = ctypes.c_int64
    lib.axon_stop_nrt_profile.argtypes = [ctypes.c_char_p]
    lib.axon_stop_nrt_profile.restype = ctypes.c_int64

    @contextlib.contextmanager
    def _hook(output_dir, device_ids):
        import jax
        jax.devices()
        if device_ids:
            ids = (ctypes.c_int64 * len(device_ids))(*device_ids)
            rc = lib.axon_start_nrt_profile(ids, len(device_ids))
        else:
            rc = lib.axon_start_nrt_profile(None, 0)
        if rc != 0:
            raise RuntimeError(f"axon_start_nrt_profile rc={rc}")
        try:
            yield
        finally:
            n = lib.axon_stop_nrt_profile(str(output_dir).encode())
            print(f"profile: {n} file(s) written to {output_dir}")

    import antenv
    mod = types.ModuleType("antenv.axon_hooks")
    mod.get_axon_ntff_profile_hook = lambda: _hook
    mod.set_axon_ntff_profile_hook = lambda h: None
    sys.modules["antenv.axon_hooks"] = mod
    antenv.axon_hooks = mod


def run_traced(x, Wsq, Wsk, Wrv, Wrq, Wrk, Wout, **kw):
    _install_ntff_shim()
    nc = _get_nc()
    in_maps = make_in_maps(x, Wsq, Wsk, Wrv, Wrq, Wrk, Wout)
    br = run_bass_kernel_spmd(nc, in_maps, list(range(NCORES)), trace=True, **kw)
    return combine(br.results), br


# revision 19
# speedup vs baseline: 1.0860x; 1.0860x over previous
"""Compositional attention Trainium2 Bass kernel (V3: HAM-warm schedule).

Sharding: 8 cores = 2 batches x 4 search-pairs.  Core c handles batch
b=c//4 and searches (2*(c%4), 2*(c%4)+1); each core produces a partial
output for its 128 rows of the S*D=512 concat dim (host sums 4 partials
per batch).

V3 notes (vs V2, 363us):
  - Wrk is folded into Wrq on the HOST (r_sim = sum_d (rq@WrkT)*retrieved),
    removing every fp32 matmul (4 cyc/row) from the device kernel.
  - x is DMA'd in 8 k-chunks and projections are ordered k,q,v,rq so the
    PE stream starts at ~3us and never gaps >3.4us: the HAM clock gate
    (K=4/8 = 1.2GHz default) un-throttles to 2.4GHz and should stay there.
  - Attention runs per-search with a software-pipelined score->exp->
    retrieve schedule (retrieval of key-pair j issues under scores of
    j+1) across an interleaved pair of query blocks.
  - Softmax denominators: flat-2D bf16 add tree (targets DVE 2x mode)
    + one ones-matmul per (si, ib); the r-softmax diff uses the folded
    rqw via one elementwise mult + a signed-ones matmul.  Both stream
    to DRAM per-ib during attention so the epilogue is short.
  - Combine uses a stacked-identity matmul whose output lands directly
    on the search's partition half (tile_position inferred), no
    partition-shift DMAs.
"""

import sys

for _p in ("/opt/trn_rl_repo",):
    if _p not in sys.path:
        sys.path.insert(0, _p)

from contextlib import ExitStack

import ml_dtypes
import numpy as np

import concourse.bass as bass
import concourse.tile as tile
from concourse import bacc
from concourse import mybir
from concourse.bass import ts
from concourse.bass_utils import run_bass_kernel_spmd
from concourse.masks import make_identity

B, N, DIM, S, R, D = 2, 2048, 1024, 8, 2, 64
NCORES = 8
SPC = 2          # searches per core
SD = SPC * D     # 128 (per-core slice of S*D)
RD = R * D       # 128
P = 128
IBL = 512        # i-block (query block)
NIB = N // IBL   # 4
KC = DIM // P    # 8
NJT = N // P     # 16 key tiles
F32 = mybir.dt.float32
BF16 = mybir.dt.bfloat16
SCALE = float(D) ** -0.5
AF = mybir.ActivationFunctionType
ALU = mybir.AluOpType


def _emit(ctx: ExitStack, tc: tile.TileContext, io):
    nc = tc.nc
    xT, wq, wk, wr, wv, wout, outp = io

    singles = ctx.enter_context(tc.tile_pool(name="singles", bufs=1))
    ident = singles.tile([P, P], BF16)
    make_identity(nc, ident)
    ones_b = singles.tile([P, 1], BF16)
    nc.vector.memset(ones_b, 1.0)
    sones = singles.tile([P, 1], BF16)
    nc.vector.memset(sones[0:64, :], 1.0)
    nc.vector.memset(sones[64:128, :], -1.0)
    # stacked identity [I64; I64] for the r-combine partition fold
    stackI = singles.tile([P, 64], BF16)
    nc.vector.tensor_tensor(stackI, ident[:, 0:64], ident[:, 64:128], ALU.add)

    # weights arrive host-prearranged as [128, KC*cols]: one contiguous DMA
    # each.  wk first on the fast queue (first projection dest), wr/wout
    # (needed last) on the slow gpsimd queue.
    wqueues = (nc.gpsimd, nc.scalar, nc.sync)
    wk_sb = singles.tile([P, KC, SD], BF16)
    wq_sb = singles.tile([P, KC, SD], BF16)
    wv_sb = singles.tile([P, KC, RD], BF16)
    wr_sb = singles.tile([P, KC, SD], BF16)
    wout_sb = singles.tile([P, DIM], BF16)
    nc.sync.dma_start(out=wk_sb, in_=wk.rearrange("p (kc m) -> p kc m", kc=KC))
    nc.scalar.dma_start(out=wq_sb, in_=wq.rearrange("p (kc m) -> p kc m", kc=KC))
    nc.scalar.dma_start(out=wv_sb, in_=wv.rearrange("p (kc m) -> p kc m", kc=KC))
    nc.gpsimd.dma_start(out=wr_sb, in_=wr.rearrange("p (kc m) -> p kc m", kc=KC))
    nc.gpsimd.dma_start(out=wout_sb, in_=wout)

    acts = ctx.enter_context(tc.tile_pool(name="acts", bufs=1))
    qT = acts.tile([P, N], BF16)
    kT = acts.tile([P, N], BF16)
    vT = acts.tile([P, N], BF16)
    rqT = acts.tile([P, N], BF16)
    vnat = acts.tile([P, NJT, RD], BF16)   # [key-part, key-tile, r*d]
    ret0 = acts.tile([P, N], BF16)         # search0 retrievedT (unnormalized)
    ret1 = acts.tile([P, N], BF16)
    rqf0 = acts.tile([P, N], BF16)         # rqw of search0 dup'd on both halves
    rqf1 = acts.tile([P, N], BF16)
    red0 = acts.tile([P, N], BF16)         # per-key-tile exp partial sums
    red1 = acts.tile([P, N], BF16)
    comp = acts.tile([P, N], BF16)         # composed output, stacked searches

    rets = (ret0, ret1)
    reds = (red0, red1)
    rqfs = (rqf0, rqf1)

    # ---------------- projections (k, q, v, rq) ----------------
    with tc.tile_pool(name="xpool", bufs=1) as xpool, \
         tc.tile_pool(name="ppsum", bufs=1, space="PSUM") as ppsum:
        xs = xpool.tile([P, KC, N], BF16)
        for k in range(KC):
            wqueues[k % 3].dma_start(out=xs[:, k, :], in_=xT[ts(k, P), :])
        # 8 psum names = 2 generations of 4 chains: dest d+1's chains never
        # wait on dest d's psum->sbuf copies
        pss8 = [ppsum.tile([P, IBL], F32, tag="pj", name=f"pj{j}")
                for j in range(8)]
        for di, (wsb, dest) in enumerate(
                ((wk_sb, kT), (wq_sb, qT), (wv_sb, vT), (wr_sb, rqT))):
            pss = pss8[4 * (di % 2):4 * (di % 2) + 4]
            for k in range(KC):
                for ib in range(NIB):
                    nc.tensor.matmul(
                        pss[ib],
                        lhsT=wsb[:, k, :],
                        rhs=xs[:, k, ts(ib, IBL)],
                        start=(k == 0),
                        stop=(k == KC - 1),
                    )
            for ib in range(NIB):
                nc.vector.tensor_copy(out=dest[:, ts(ib, IBL)], in_=pss[ib])
                if dest is vT:
                    # v to natural [keys, r*d] layout via DMA transposes
                    # (keeps the PE stream matmul-only)
                    for h in range(4):
                        jt = 4 * ib + h
                        tq = nc.scalar if jt % 2 == 0 else nc.sync
                        tq.dma_start_transpose(
                            out=vnat[:, jt, :], in_=vT[:, ts(jt, P)])
        # duplicate each search's rqw onto both partition halves
        for si in range(SPC):
            lo = 64 * si
            nc.gpsimd.dma_start(out=rqfs[si][0:64, :],
                                in_=rqT[lo:lo + 64, :])
            nc.gpsimd.dma_start(out=rqfs[si][64:128, :],
                                in_=rqT[lo:lo + 64, :])

    # DRAM bounce buffers for per-query scalars ([1,N] <-> [128,N/128])
    dramp = ctx.enter_context(tc.tile_pool(name="dramp", bufs=1, space="DRAM"))
    diff_dr = [dramp.tile([N], F32, tag=f"diff{si}", name=f"diff{si}")
               for si in range(SPC)]
    sums_dr = [dramp.tile([N], F32, tag=f"sums{si}", name=f"sums{si}")
               for si in range(SPC)]
    ra0_dr = [dramp.tile([N], BF16, tag=f"ra0{si}", name=f"ra0d{si}")
              for si in range(SPC)]
    ra1_dr = [dramp.tile([N], BF16, tag=f"ra1{si}", name=f"ra1d{si}")
              for si in range(SPC)]

    # pools that live from attention to the end
    epsum = ctx.enter_context(tc.tile_pool(name="epsum", bufs=2, space="PSUM"))
    etmp = ctx.enter_context(tc.tile_pool(name="etmp", bufs=2))
    btmp = ctx.enter_context(tc.tile_pool(name="btmp", bufs=1))

    dmults = {}

    def sums_diff_emit(si, ib):
        """Partition-reduce the exp sums and the rqw*ret diff for one
        (search, query-block); stream both to DRAM.  The elementwise
        dmult was computed at pair-end so the matmuls here have
        long-ready deps and never stall the in-order PE stream."""
        psm = epsum.tile([P, IBL], F32, tag="ep", name="ep")
        nc.tensor.matmul(psm[:1, :], lhsT=ones_b,
                         rhs=reds[si][:, ts(ib, IBL)], start=True, stop=True)
        smst = etmp.tile([1, IBL], F32, tag="smst")
        nc.scalar.copy(out=smst, in_=psm[:1, :])
        nc.sync.dma_start(out=sums_dr[si][None, ts(ib, IBL)], in_=smst[0:1, :])

        pdm = epsum.tile([P, IBL], F32, tag="ep", name="ep")
        nc.tensor.matmul(pdm[:1, :], lhsT=sones, rhs=dmults[(si, ib)],
                         start=True, stop=True)
        pdst = etmp.tile([1, IBL], F32, tag="pdst")
        nc.scalar.copy(out=pdst, in_=pdm[:1, :])
        nc.sync.dma_start(out=diff_dr[si][None, ts(ib, IBL)], in_=pdst[0:1, :])

    def epilogue_emit(si):
        """r-softmax weights for one search -> broadcast -> combined
        output rows land on comp[64*si : 64*si+64]."""
        d128 = etmp.tile([P, N // P], F32, tag="d128")
        s128 = etmp.tile([P, N // P], F32, tag="s128")
        nc.gpsimd.dma_start(out=d128,
                            in_=diff_dr[si].rearrange("(p f) -> p f", p=P))
        nc.gpsimd.dma_start(out=s128,
                            in_=sums_dr[si].rearrange("(p f) -> p f", p=P))
        inv = etmp.tile([P, N // P], F32, tag="inv")
        nc.vector.reciprocal(inv, s128)
        targ = etmp.tile([P, N // P], F32, tag="targ")
        nc.vector.tensor_tensor(targ, d128, inv, ALU.mult)
        sig = etmp.tile([P, N // P], F32, tag="sig")
        nc.scalar.activation(out=sig, in_=targ, func=AF.Sigmoid, scale=SCALE)
        isig = etmp.tile([P, N // P], F32, tag="isig")
        nc.scalar.activation(out=isig, in_=targ, func=AF.Sigmoid, scale=-SCALE)
        ra0 = etmp.tile([P, N // P], BF16, tag="ra0")
        nc.vector.tensor_tensor(ra0, sig, inv, ALU.mult)
        ra1 = etmp.tile([P, N // P], BF16, tag="ra1")
        nc.vector.tensor_tensor(ra1, isig, inv, ALU.mult)
        nc.gpsimd.dma_start(out=ra0_dr[si].rearrange("(p f) -> p f", p=P),
                            in_=ra0)
        nc.gpsimd.dma_start(out=ra1_dr[si].rearrange("(p f) -> p f", p=P),
                            in_=ra1)
        rab = btmp.tile([P, N], BF16, tag=f"rab{si}", name=f"rab{si}")
        nc.gpsimd.dma_start(out=rab[0:64, :],
                            in_=ra0_dr[si][None, :].to_broadcast([64, N]))
        nc.gpsimd.dma_start(out=rab[64:128, :],
                            in_=ra1_dr[si][None, :].to_broadcast([64, N]))
        tmp = btmp.tile([P, N], BF16, tag=f"tmp{si}", name=f"tmp{si}")
        nc.vector.tensor_tensor(tmp, rab, rets[si], ALU.mult)
        lo = 64 * si
        for h in range(NIB):
            cp = epsum.tile([P, IBL], F32, tag="ep", name="ep")
            nc.tensor.matmul(cp[lo:lo + 64, :], lhsT=stackI,
                             rhs=tmp[:, ts(h, IBL)], start=True, stop=True)
            nc.scalar.copy(out=comp[lo:lo + 64, ts(h, IBL)],
                           in_=cp[lo:lo + 64, :])

    # ---------------- attention (per search, ib-pairs) ----------------
    with tc.tile_pool(name="expp", bufs=2) as expp, \
         tc.tile_pool(name="trp1", bufs=2) as trp1, \
         tc.tile_pool(name="trp2", bufs=2) as trp2, \
         tc.tile_pool(name="trp3", bufs=2) as trp3, \
         tc.tile_pool(name="scp", bufs=2, space="PSUM") as scp, \
         tc.tile_pool(name="mps", bufs=1, space="PSUM") as mps:
        pending = []       # deferred sums/diff emits to slot into PE stream
        for si in range(SPC):
            lo = 64 * si
            for ibp in range(NIB // 2):
                ibs = (2 * ibp, 2 * ibp + 1)
                ets = {ib: expp.tile([P, NJT, IBL], BF16, tag="exp",
                                     name=f"exp{ib % 2}")
                       for ib in ibs}
                rt = {ib: mps.tile([P, IBL], F32, tag="mm",
                                   name=f"rt{ib % 2}")
                      for ib in ibs}
                for jg in range(NJT // 2 + 1):
                    # mid-pair: emit deferred work so its matmuls slot
                    # into a busy PE stream (deps are long since ready)
                    if jg in (3, 7) and pending:
                        fn, args = pending.pop(0)
                        fn(*args)
                    if si == 1 and ibp == 1 and jg == 3:
                        epilogue_emit(0)
                    for ib in ibs:
                        if jg < NJT // 2:
                            for h in range(2):
                                jt = 2 * jg + h
                                sp = scp.tile([P, IBL], F32, tag="sc",
                                              name=f"sc{ib % 2}")
                                nc.tensor.matmul(
                                    sp,
                                    lhsT=kT[lo:lo + 64, ts(jt, P)],
                                    rhs=qT[lo:lo + 64, ts(ib, IBL)],
                                    start=True, stop=True,
                                )
                                nc.scalar.activation(
                                    out=ets[ib][:, jt, :], in_=sp,
                                    func=AF.Exp, scale=SCALE,
                                )
                        if jg > 0:
                            for h in range(2):
                                jt = 2 * (jg - 1) + h
                                nc.tensor.matmul(
                                    rt[ib], lhsT=vnat[:, jt, :],
                                    rhs=ets[ib][:, jt, :],
                                    start=(jt == 0), stop=(jt == NJT - 1),
                                    skip_group_check=True,
                                )
                for ib in ibs:
                    nc.vector.tensor_copy(out=rets[si][:, ts(ib, IBL)],
                                          in_=rt[ib])
                    # flat-2D bf16 add tree over the 16 key tiles
                    etf = ets[ib].rearrange("p a b -> p (a b)")
                    g1 = trp1.tile([P, NJT // 2 * IBL], BF16, tag="g1")
                    nc.vector.tensor_tensor(g1, etf[:, 0:4096],
                                            etf[:, 4096:8192], ALU.add)
                    g2 = trp2.tile([P, NJT // 4 * IBL], BF16, tag="g2")
                    nc.vector.tensor_tensor(g2, g1[:, 0:2048],
                                            g1[:, 2048:4096], ALU.add)
                    g3 = trp3.tile([P, NJT // 8 * IBL], BF16, tag="g3")
                    nc.vector.tensor_tensor(g3, g2[:, 0:1024],
                                            g2[:, 1024:2048], ALU.add)
                    nc.gpsimd.tensor_tensor(reds[si][:, ts(ib, IBL)],
                                            g3[:, 0:512], g3[:, 512:1024],
                                            ALU.add)
                    dm = etmp.tile([P, IBL], BF16, tag="dmult",
                                   name=f"dmult{ib % 2}")
                    nc.vector.tensor_tensor(dm, rqfs[si][:, ts(ib, IBL)],
                                            rets[si][:, ts(ib, IBL)],
                                            ALU.mult)
                    dmults[(si, ib)] = dm
                    pending.append((sums_diff_emit, (si, ib)))

    # ---------------- tail: drain + epilogue(1) + output projection ----
    with tc.tile_pool(name="warmp", bufs=1, space="PSUM") as warmp, \
         tc.tile_pool(name="opo", bufs=2, space="PSUM") as opo:
        scratch = warmp.tile([P, IBL], F32, tag="warm")

        def filler(n):
            # redundant matmuls into a never-read scratch bank: keep the
            # HAM activity window busy so the PE doesn't re-throttle
            # while the epilogue's scalar dance resolves
            for f in range(n):
                nc.tensor.matmul(scratch, lhsT=wout_sb[:, 0:P],
                                 rhs=qT[:, ts(f % 4, IBL)],
                                 start=True, stop=True)

        for fn, args in pending:
            fn(*args)
            filler(2)
        filler(8)
        epilogue_emit(1)

        for nch in range(N // P):
            for h in range(DIM // IBL):
                j = 2 * nch + h
                pw = opo.tile([P, IBL], F32, tag="pw", name=f"pw{j % 2}")
                nc.tensor.matmul(pw, lhsT=comp[:, ts(nch, P)],
                                 rhs=wout_sb[:, ts(h, IBL)],
                                 start=True, stop=True)
                owst = etmp.tile([P, IBL], F32, tag="owst",
                                 name=f"owst{j % 2}")
                if j % 2 == 0:
                    nc.scalar.copy(out=owst, in_=pw)
                else:
                    nc.vector.tensor_copy(out=owst, in_=pw)
                wqueues[j % 3].dma_start(out=outp[ts(nch, P), ts(h, IBL)],
                                         in_=owst)


def build_nc():
    nc = bacc.Bacc()
    xT = nc.declare_dram_parameter("xT", [DIM, N], BF16, isOutput=False)
    # weights host-prearranged to [128, KC*cols] (partition-major chunks)
    wq = nc.declare_dram_parameter("wq", [P, KC * SD], BF16, isOutput=False)
    wk = nc.declare_dram_parameter("wk", [P, KC * SD], BF16, isOutput=False)
    wr = nc.declare_dram_parameter("wr", [P, KC * SD], BF16, isOutput=False)
    wv = nc.declare_dram_parameter("wv", [P, KC * RD], BF16, isOutput=False)
    wout = nc.declare_dram_parameter("wout", [SD, DIM], BF16, isOutput=False)
    outp = nc.declare_dram_parameter("outp", [N, DIM], F32, isOutput=True)
    io = (xT[:], wq[:], wk[:], wr[:], wv[:], wout[:], outp[:])
    with tile.TileContext(nc) as tc:
        with ExitStack() as ctx:
            _emit(ctx, tc, io)
    nc.compile()
    return nc


_CACHE = {}


def _get_nc():
    if "nc" not in _CACHE:
        _CACHE["nc"] = build_nc()
    return _CACHE["nc"]


def make_in_maps(x, Wsq, Wsk, Wrv, Wrq, Wrk, Wout):
    x = np.asarray(x, np.float32)
    bf = ml_dtypes.bfloat16
    # fold Wrk into Wrq:  r_sim = sum_d (rq @ Wrk^T) * retrieved
    wr_folded = (
        np.asarray(Wrq, np.float32).reshape(DIM, S, D)
        @ np.asarray(Wrk, np.float32).T
    ).reshape(DIM, S * D)

    def prearrange(w):
        # [DIM, cols] -> [128, KC*cols]: partition-major k-chunks so the
        # device load is one contiguous DMA
        cols = w.shape[1]
        return np.ascontiguousarray(
            w.reshape(KC, P, cols).transpose(1, 0, 2).reshape(P, KC * cols)
        ).astype(bf)

    in_maps = []
    for c in range(NCORES):
        b = c // 4
        s0 = 2 * (c % 4)
        sl = slice(s0 * D, (s0 + 2) * D)
        in_maps.append({
            "xT": np.ascontiguousarray(x[b].T).astype(bf),
            "wq": prearrange(np.asarray(Wsq, np.float32)[:, sl]),
            "wk": prearrange(np.asarray(Wsk, np.float32)[:, sl]),
            "wr": prearrange(wr_folded[:, sl]),
            "wv": prearrange(np.asarray(Wrv, np.float32)),
            "wout": np.ascontiguousarray(np.asarray(Wout, np.float32)[sl, :]).astype(bf),
        })
    return in_maps


def combine(results):
    out = np.zeros((B, N, DIM), np.float32)
    for c in range(NCORES):
        out[c // 4] += np.asarray(results[c]["outp"], np.float32)
    return out


def kernel(x, Wsq, Wsk, Wrv, Wrq, Wrk, Wout):
    nc = _get_nc()
    in_maps = make_in_maps(x, Wsq, Wsk, Wrv, Wrq, Wrk, Wout)
    res = run_bass_kernel_spmd(nc, in_maps, list(range(NCORES))).results
    return combine(res)


def _install_ntff_shim():
    """Provide antenv.axon_hooks in images that lack it, driving NTFF
    profiling via ctypes into the injected libaxon_pjrt.so."""
    import types
    import ctypes
    import contextlib

    try:
        from antenv.axon_hooks import get_axon_ntff_profile_hook  # noqa
        return
    except ImportError:
        pass
    so_path = "/opt/axon/libaxon_pjrt.so"
    lib = ctypes.CDLL(so_path)
    if not hasattr(lib, "axon_start_nrt_profile"):
        return
    lib.axon_start_nrt_profile.argtypes = [
        ctypes.POINTER(ctypes.c_int64), ctypes.c_size_t]
    lib.axon_start_nrt_profile.restype = ctypes.c_int64
    lib.axon_stop_nrt_profile.argtypes = [ctypes.c_char_p]
    lib.axon_stop_nrt_profile.restype = ctypes.c_int64

    @contextlib.contextmanager
    def _hook(output_dir, device_ids):
        import jax
        jax.devices()
        if device_ids:
            ids = (ctypes.c_int64 * len(device_ids))(*device_ids)
            rc = lib.axon_start_nrt_profile(ids, len(device_ids))
        else:
            rc = lib.axon_start_nrt_profile(None, 0)
        if rc != 0:
            raise RuntimeError(f"axon_start_nrt_profile rc={rc}")
        try:
            yield
        finally:
            n = lib.axon_stop_nrt_profile(str(output_dir).encode())
            print(f"profile: {n} file(s) written to {output_dir}")

    import antenv
    mod = types.ModuleType("antenv.axon_hooks")
    mod.get_axon_ntff_profile_hook = lambda: _hook
    mod.set_axon_ntff_profile_hook = lambda h: None
    sys.modules["antenv.axon_hooks"] = mod
    antenv.axon_hooks = mod


def run_traced(x, Wsq, Wsk, Wrv, Wrq, Wrk, Wout, **kw):
    _install_ntff_shim()
    nc = _get_nc()
    in_maps = make_in_maps(x, Wsq, Wsk, Wrv, Wrq, Wrk, Wout)
    br = run_bass_kernel_spmd(nc, in_maps, list(range(NCORES)), trace=True, **kw)
    return combine(br.results), br
